# revision 4
# baseline (speedup 1.0000x reference)
"""Llama attention layer on 8 Trainium2 NeuronCores (tensor-parallel over heads).

Sharding: each core owns 2 of 16 heads. wq/wk/wv column-sharded, wo row-sharded.
x is replicated; the o_proj partial outputs are summed on the host (the
"all-reduce" of the row-parallel output).

On-device layout is fully transposed ("feature-major") so that no transposes
are needed anywhere:
  - xT        [d, tok]      d on partitions
  - qT, kT    [j', tok]     j' = per-head feature, parity-major (RoPE perm)
  - scoresT   [t, s]        from matmul(lhsT=kT tile, rhs=qT tile)
  - expT      [t, s]        exp on ACT; causal mask = multiply by exp(mask)
  - outT      [j, s]        from matmul(lhsT=v tile [t, j], rhs=expT)
  - y         [s, e]        from matmul(lhsT=outT tile, rhs=woT)
Softmax denominator = ones-row matmul over expT (partition reduction on PE),
broadcast back across partitions with a K=1 matmul, reciprocal on DVE, and
folded into the outT psum eviction. No max-subtraction: |scores| is O(5) for
this distribution and exp is computed in fp32 from the fp32 psum.
"""

import math
import os

import numpy as np
import ml_dtypes

import concourse.bass as bass
import concourse.tile as tile
from concourse import bacc, mybir
from concourse.bass_utils import run_bass_kernel_spmd
from contextlib import ExitStack

BF16 = mybir.dt.bfloat16
F32 = mybir.dt.float32
AF = mybir.ActivationFunctionType

N_CORES = 8
B, S, D = 2, 2048, 2048
H = 16                      # total heads
HPC = H // N_CORES          # heads per core = 2
HD = D // H                 # head dim = 128
EC = HPC * HD               # features per core = 256
TOK = B * S                 # 4096
P = 128
NDT = D // P                # 16 d-tiles
NTB = TOK // 512            # 8 tok blocks of 512
NSB = S // 512              # 4 s-blocks per batch
NTT = S // P                # 16 t-tiles per batch
SCALE = 1.0 / math.sqrt(HD)

ts = bass.ts
ds = bass.ds

LAST_EXEC_NS = None
TRACE = bool(int(os.environ.get("KERNEL_TRACE", "0")))
BACKEND = os.environ.get("KERNEL_BACKEND", "hw")  # "hw" | "sim"

_PROGRAM_CACHE = {}


def _install_trace_hook():
    """Register an NTFF-profile hook for trace=True under axon when the
    image's antenv lacks axon_hooks (replicates trn_boot's ctypes shim)."""
    import sys as _sys
    import types
    import ctypes
    import contextlib

    try:
        from antenv.axon_hooks import get_axon_ntff_profile_hook  # noqa: F401
        return True
    except ImportError:
        pass

    so_path = "/opt/axon/libaxon_pjrt.so"
    if not os.path.exists(so_path):
        return False
    lib = ctypes.CDLL(so_path)
    if not hasattr(lib, "axon_start_nrt_profile"):
        return False
    lib.axon_start_nrt_profile.argtypes = [
        ctypes.POINTER(ctypes.c_int64),
        ctypes.c_size_t,
    ]
    lib.axon_start_nrt_profile.restype = ctypes.c_int64
    lib.axon_stop_nrt_profile.argtypes = [ctypes.c_char_p]
    lib.axon_stop_nrt_profile.restype = ctypes.c_int64

    @contextlib.contextmanager
    def _hook(output_dir, device_ids):
        import jax
        jax.devices()
        if device_ids:
            ids = (ctypes.c_int64 * len(device_ids))(*device_ids)
            rc = lib.axon_start_nrt_profile(ids, len(device_ids))
        else:
            rc = lib.axon_start_nrt_profile(None, 0)
        if rc != 0:
            raise RuntimeError(f"axon_start_nrt_profile rc={rc}")
        try:
            yield
        finally:
            n = lib.axon_stop_nrt_profile(str(output_dir).encode())
            print(f"profile: {n} file(s) written to {output_dir}")

    import antenv
    mod = types.ModuleType("antenv.axon_hooks")
    mod._hook = _hook
    mod.get_axon_ntff_profile_hook = lambda: _hook
    mod.set_axon_ntff_profile_hook = lambda h: None
    _sys.modules["antenv.axon_hooks"] = mod
    antenv.axon_hooks = mod

    # artifact upload has no bucket access in this container; stub it
    import concourse.bass_utils as _bu
    _bu.upload_artifacts = lambda tmpdir: f"local://{tmpdir}"
    return True


def _classify_mask(mask):
    """Split the [S, S] additive mask into [t-128 x s-512] blocks per s-block.

    Returns (blocks, pats): blocks[m] = list of (j, pat_id|None) t-tiles to
    process for s-block m; pats = list of [128, 512] fp32 exp(mask) patterns.
    """
    mm = np.asarray(mask, np.float32).reshape(S, S)
    pats = []
    pat_ids = {}
    blocks = []
    for m in range(NSB):
        lst = []
        for j in range(NTT):
            blk = mm[m * 512:(m + 1) * 512, j * P:(j + 1) * P]  # [s, t]
            if np.all(blk <= -30.0):
                continue  # exp == 0: contributes nothing to av or den
            if np.all(blk == 0.0):
                lst.append((j, None))
                continue
            pt = np.exp(np.minimum(blk.T, 80.0)).astype(np.float32)  # [t, s]
            key = pt.tobytes()
            if key not in pat_ids:
                pat_ids[key] = len(pats)
                pats.append(pt)
            lst.append((j, pat_ids[key]))
        blocks.append(lst)
    return blocks, pats


def _emit(ctx, tc, io, blocks, npat):
    nc = tc.nc

    const = ctx.enter_context(tc.tile_pool(name="const", bufs=1))
    persist = ctx.enter_context(tc.tile_pool(name="persist", bufs=1))
    xt_pool = ctx.enter_context(tc.tile_pool(name="xt_pool", bufs=2))
    rope_pool = ctx.enter_context(tc.tile_pool(name="rope_pool", bufs=2))
    exp_pool = ctx.enter_context(tc.tile_pool(name="exp_pool", bufs=18))
    den_pool = ctx.enter_context(tc.tile_pool(name="den_pool", bufs=8))
    recip_pool = ctx.enter_context(tc.tile_pool(name="recip_pool", bufs=3))
    y_pool = ctx.enter_context(tc.tile_pool(name="y_pool", bufs=3))
    psum_mm = ctx.enter_context(tc.tile_pool(name="psum_mm", bufs=4, space="PSUM"))
    psum_acc = ctx.enter_context(tc.tile_pool(name="psum_acc", bufs=2, space="PSUM"))
    psum_den = ctx.enter_context(tc.tile_pool(name="psum_den", bufs=2, space="PSUM"))

    # --- constants / weights ---
    wq_sb = const.tile([P, NDT, HPC, P], BF16)
    nc.sync.dma_start(wq_sb[:], io["wqt"][:])
    wk_sb = const.tile([P, NDT, HPC, P], BF16)
    nc.sync.dma_start(wk_sb[:], io["wkt"][:])
    wv_sb = const.tile([P, NDT, EC], BF16)
    nc.sync.dma_start(wv_sb[:], io["wvt"][:])
    wo_sb = const.tile([P, HPC, D], BF16)
    cos_sb = const.tile([P, TOK], BF16)
    nc.sync.dma_start(cos_sb[:], io["cos2"][:])
    sin_sb = const.tile([P, TOK], BF16)
    nc.sync.dma_start(sin_sb[:], io["sin2"][:])
    pat_sb = const.tile([P, npat, 512], BF16)
    ones_col = const.tile([P, 1], BF16)
    nc.any.memset(ones_col[:], 1.0)
    ones_row = const.tile([1, P], BF16)
    nc.any.memset(ones_row[:], 1.0)

    q_sb = persist.tile([P, HPC, TOK], BF16)   # [parity*64+i, h, tok]
    k_sb = persist.tile([P, HPC, TOK], BF16)
    v_sb = persist.tile([P, TOK // P, EC], BF16)  # [t%128, t-tile, (h, j)]
    outT_sb = persist.tile([P, B * HPC, S], BF16)  # [j, pair, s]

    # --- phase 1: q/k/v projections + RoPE, per 512-token block ---
    for tb in range(NTB):
        xt_t = xt_pool.tile([P, NDT, 512], BF16)
        nc.sync.dma_start(xt_t[:], io["xt"][:, :, ts(tb, 512)])

        for w_sb, dst in ((wq_sb, q_sb), (wk_sb, k_sb)):
            for h in range(HPC):
                qk_ps = psum_mm.tile([P, 512], F32, tag="mm")
                for dt in range(NDT):
                    nc.tensor.matmul(
                        qk_ps[:], lhsT=w_sb[:, dt, h, :], rhs=xt_t[:, dt, :],
                        start=(dt == 0), stop=(dt == NDT - 1),
                    )
                nc.vector.tensor_copy(dst[:, h, ts(tb, 512)], qk_ps[:])

        for q4 in range(4):
            v_ps = psum_mm.tile([P, EC], F32, tag="mm")
            for dt in range(NDT):
                nc.tensor.matmul(
                    v_ps[:], lhsT=xt_t[:, dt, ts(q4, P)], rhs=wv_sb[:, dt, :],
                    start=(dt == 0), stop=(dt == NDT - 1),
                )
            nc.vector.tensor_copy(v_sb[:, tb * 4 + q4, :], v_ps[:])

        # RoPE on q and k for this token block (parity-major feature order:
        # partitions 0:64 hold even features t0, 64:128 hold odd t1).
        for a_sb in (q_sb, k_sb):
            swp = rope_pool.tile([P, HPC, 512], BF16, tag="swp")
            nc.sync.dma_start(swp[0:64, :, :], a_sb[64:128, :, ts(tb, 512)])
            nc.sync.dma_start(swp[64:128, :, :], a_sb[0:64, :, ts(tb, 512)])
            for h in range(HPC):
                r1 = rope_pool.tile([P, 512], BF16, tag="r1")
                nc.vector.tensor_mul(r1[:], a_sb[:, h, ts(tb, 512)], cos_sb[:, ts(tb, 512)])
                r2 = rope_pool.tile([P, 512], BF16, tag="r2")
                nc.vector.tensor_mul(r2[:], swp[:, h, :], sin_sb[:, ts(tb, 512)])
                nc.vector.tensor_add(a_sb[:, h, ts(tb, 512)], r1[:], r2[:])

    nc.sync.dma_start(pat_sb[:], io["pat"][:])

    # --- phase 2: attention per (batch, head) pair, per 512-query block ---
    den_rows = []
    for b in range(B):
        for h in range(HPC):
            pi = b * HPC + h
            for m in range(NSB):
                s_sl = ds(b * S + m * 512, 512)
                tlist = blocks[m]
                n_t = len(tlist)
                av_ps = psum_acc.tile([P, 512], F32, tag="acc")
                den_ps = psum_den.tile([1, 512], F32, tag="den")
                # scores + exp first (PE runs ahead of ACT through the psum
                # pool); den/av matmuls afterwards so PE never waits on exp.
                exs = []
                for j, pid in tlist:
                    sc_ps = psum_mm.tile([P, 512], F32, tag="mm")
                    nc.tensor.matmul(
                        sc_ps[:], lhsT=k_sb[:, h, ds(b * S + j * P, P)],
                        rhs=q_sb[:, h, s_sl], start=True, stop=True,
                    )
                    ex = exp_pool.tile([P, 512], BF16, tag="ex")
                    nc.scalar.activation(ex[:], sc_ps[:], AF.Exp, scale=SCALE)
                    if pid is not None:
                        nc.vector.tensor_mul(ex[:], ex[:], pat_sb[:, pid, :])
                    exs.append(ex)
                for idx, ((j, pid), ex) in enumerate(zip(tlist, exs)):
                    nc.tensor.matmul(
                        den_ps[:], lhsT=ones_col[:], rhs=ex[:],
                        start=(idx == 0), stop=(idx == n_t - 1),
                    )
                    nc.tensor.matmul(
                        av_ps[:], lhsT=v_sb[:, b * NTT + j, ds(h * HD, HD)],
                        rhs=ex[:], start=(idx == 0), stop=(idx == n_t - 1),
                    )
                # evict UNNORMALIZED output; normalization is deferred off the
                # PE critical path so consecutive blocks' matmuls stay dense.
                den_row = den_pool.tile([1, 512], BF16, tag="denr")
                nc.scalar.copy(den_row[:], den_ps[:])
                nc.vector.tensor_copy(outT_sb[:, pi, ds(m * 512, 512)], av_ps[:])
                den_rows.append((pi, m, den_row))
            # normalize this pair's blocks; overlaps the next pair's attention
            for pi2, m2, dr in den_rows:
                bc_ps = psum_mm.tile([P, 512], F32, tag="mm")
                nc.tensor.matmul(bc_ps[:], lhsT=ones_row[:], rhs=dr[:],
                                 start=True, stop=True)
                rc = recip_pool.tile([P, 512], F32, tag="rc")
                nc.vector.reciprocal_approx_fast(rc[:], bc_ps[:])
                sl2 = ds(m2 * 512, 512)
                nc.vector.tensor_mul(outT_sb[:, pi2, sl2], outT_sb[:, pi2, sl2], rc[:])
            den_rows = []

    nc.sync.dma_start(wo_sb[:], io["wot"][:])

    # --- phase 3: o_proj partial: y[s, e] = sum_h outT_h^T @ woT_h ---
    for st in range(TOK // P):
        b = st // NTT
        sl = st % NTT
        for eb in range(D // 512):
            y_ps = psum_mm.tile([P, 512], F32, tag="mm")
            for h in range(HPC):
                nc.tensor.matmul(
                    y_ps[:], lhsT=outT_sb[:, b * HPC + h, ts(sl, P)],
                    rhs=wo_sb[:, h, ts(eb, 512)],
                    start=(h == 0), stop=(h == HPC - 1),
                )
            y_sb = y_pool.tile([P, 512], BF16, tag="y")
            if (st * (D // 512) + eb) % 2 == 0:
                nc.scalar.copy(y_sb[:], y_ps[:])
            else:
                nc.vector.tensor_copy(y_sb[:], y_ps[:])
            nc.sync.dma_start(io["y"][st, :, ts(eb, 512)], y_sb[:])


def _build_program(blocks_key, blocks, npat):
    nc = bacc.Bacc(
        "TRN2", target_bir_lowering=False, debug=False, enable_asserts=False
    )
    io = {
        "xt": nc.dram_tensor("xt", [P, NDT, TOK], BF16, kind="ExternalInput").ap(),
        "wqt": nc.dram_tensor("wqt", [P, NDT, HPC, P], BF16, kind="ExternalInput").ap(),
        "wkt": nc.dram_tensor("wkt", [P, NDT, HPC, P], BF16, kind="ExternalInput").ap(),
        "wvt": nc.dram_tensor("wvt", [P, NDT, EC], BF16, kind="ExternalInput").ap(),
        "wot": nc.dram_tensor("wot", [P, HPC, D], BF16, kind="ExternalInput").ap(),
        "cos2": nc.dram_tensor("cos2", [P, TOK], BF16, kind="ExternalInput").ap(),
        "sin2": nc.dram_tensor("sin2", [P, TOK], BF16, kind="ExternalInput").ap(),
        "pat": nc.dram_tensor("pat", [P, npat, 512], BF16, kind="ExternalInput").ap(),
        "y": nc.dram_tensor("y", [TOK // P, P, D], BF16, kind="ExternalOutput").ap(),
    }
    with tile.TileContext(nc) as tc:
        with ExitStack() as ctx:
            _emit(ctx, tc, io, blocks, npat)
    nc.compile()
    return nc


def _get_program(mask):
    blocks, pats = _classify_mask(mask)
    key = tuple(tuple(b) for b in blocks)
    if key not in _PROGRAM_CACHE:
        npat = max(len(pats), 1)
        nc = _build_program(key, blocks, npat)
        _PROGRAM_CACHE[key] = (nc, npat)
    nc, npat = _PROGRAM_CACHE[key]
    pat_np = np.zeros((P, npat, 512), np.float32)
    for i, pt in enumerate(pats):
        pat_np[:, i, :] = pt
    return nc, pat_np


def _bf16(a):
    return np.asarray(a, np.float32).astype(ml_dtypes.bfloat16)


def kernel(x, wq, wk, wv, wo, freqs_cos, freqs_sin, mask):
    global LAST_EXEC_NS
    x = np.asarray(x, np.float32)
    wq = np.asarray(wq, np.float32)
    wk = np.asarray(wk, np.float32)
    wv = np.asarray(wv, np.float32)
    wo = np.asarray(wo, np.float32)
    freqs_cos = np.asarray(freqs_cos, np.float32)
    freqs_sin = np.asarray(freqs_sin, np.float32)

    nc, pat_np = _get_program(mask)

    # xT: [d, tok] -> [dp, dt, tok]
    xt = _bf16(
        x.reshape(TOK, D).T.reshape(NDT, P, TOK).transpose(1, 0, 2)
    )

    # cos/sin, parity-major RoPE operands: [128, tok]
    cosT = np.tile(freqs_cos.T, (1, B))          # [64, TOK]
    sinT = np.tile(freqs_sin.T, (1, B))
    cos2 = _bf16(np.concatenate([cosT, cosT], axis=0))
    sin2 = _bf16(np.concatenate([-sinT, sinT], axis=0))
    pat = _bf16(pat_np)

    # per-head parity-major row permutation for q/k weights
    perm1 = np.r_[np.arange(0, P, 2), np.arange(1, P, 2)]

    in_maps = []
    for c in range(N_CORES):
        rows = slice(c * EC, (c + 1) * EC)
        wq_c, wk_c, wv_c = wq[rows], wk[rows], wv[rows]   # [256, D]
        wo_c = wo[:, rows]                                # [D, 256]
        row_perm = np.concatenate([h * P + perm1 for h in range(HPC)])
        wqt = _bf16(wq_c[row_perm].T.reshape(NDT, P, HPC, P).transpose(1, 0, 2, 3))
        wkt = _bf16(wk_c[row_perm].T.reshape(NDT, P, HPC, P).transpose(1, 0, 2, 3))
        wvt = _bf16(wv_c.T.reshape(NDT, P, EC).transpose(1, 0, 2))
        wot = _bf16(wo_c.T.reshape(HPC, P, D).transpose(1, 0, 2))
        in_maps.append({
            "xt": xt, "wqt": wqt, "wkt": wkt, "wvt": wvt, "wot": wot,
            "cos2": cos2, "sin2": sin2, "pat": pat,
        })

    if BACKEND == "sim":
        from concourse.bass_interp import CoreSim
        results = []
        for c in range(N_CORES):
            sim = CoreSim(nc, trace=False)
            for name, arr in in_maps[c].items():
                sim.tensor(name)[:] = arr
            sim.tensor("y")[:] = 0
            sim.simulate()
            results.append({"y": np.array(sim.tensor("y"))})
    else:
        do_trace = TRACE and _install_trace_hook()
        res = run_bass_kernel_spmd(
            nc, in_maps, core_ids=list(range(N_CORES)), trace=do_trace,
        )
        results = res.results
        LAST_EXEC_NS = res.exec_time_ns

    y = np.zeros((TOK, D), np.float32)
    for c in range(N_CORES):
        y += results[c]["y"].reshape(TOK, D).astype(np.float32)
    return y.reshape(B, S, D)



# revision 10
# speedup vs baseline: 1.0213x; 1.0213x over previous
"""Llama attention layer on 8 Trainium2 NeuronCores (tensor-parallel over heads).

Sharding: each core owns 2 of 16 heads. wq/wk/wv column-sharded, wo row-sharded.
x is replicated; the o_proj partial outputs are summed on the host (the
"all-reduce" of the row-parallel output).

On-device layout is fully transposed ("feature-major") so that no transposes
are needed anywhere:
  - xT        [d, tok]      d on partitions
  - qT, kT    [j', tok]     j' = per-head feature, parity-major (RoPE perm)
  - scoresT   [t, s]        from matmul(lhsT=kT tile, rhs=qT tile)
  - expT      [t, s]        exp on ACT; causal mask = multiply by exp(mask)
  - outT      [j, s]        from matmul(lhsT=v tile [t, j], rhs=expT)
  - y         [s, e]        from matmul(lhsT=outT tile, rhs=woT)
Softmax denominator = ones-row matmul over expT (partition reduction on PE),
broadcast back across partitions with a K=1 matmul, reciprocal on DVE, and
folded into the outT psum eviction. No max-subtraction: |scores| is O(5) for
this distribution and exp is computed in fp32 from the fp32 psum.
"""

import math
import os

import numpy as np
import ml_dtypes

import concourse.bass as bass
import concourse.tile as tile
from concourse import bacc, mybir
from concourse.bass_utils import run_bass_kernel_spmd
from contextlib import ExitStack

BF16 = mybir.dt.bfloat16
F32 = mybir.dt.float32
AF = mybir.ActivationFunctionType

N_CORES = 8
B, S, D = 2, 2048, 2048
H = 16                      # total heads
HPC = H // N_CORES          # heads per core = 2
HD = D // H                 # head dim = 128
EC = HPC * HD               # features per core = 256
TOK = B * S                 # 4096
P = 128
NDT = D // P                # 16 d-tiles
NTB = TOK // 512            # 8 tok blocks of 512
NSB = S // 512              # 4 s-blocks per batch
NTT = S // P                # 16 t-tiles per batch
SCALE = 1.0 / math.sqrt(HD)

ts = bass.ts
ds = bass.ds

LAST_EXEC_NS = None
TRACE = bool(int(os.environ.get("KERNEL_TRACE", "0")))
BACKEND = os.environ.get("KERNEL_BACKEND", "hw")  # "hw" | "sim"

_PROGRAM_CACHE = {}


def _install_trace_hook():
    """Register an NTFF-profile hook for trace=True under axon when the
    image's antenv lacks axon_hooks (replicates trn_boot's ctypes shim)."""
    import sys as _sys
    import types
    import ctypes
    import contextlib

    try:
        from antenv.axon_hooks import get_axon_ntff_profile_hook  # noqa: F401
        return True
    except ImportError:
        pass

    so_path = "/opt/axon/libaxon_pjrt.so"
    if not os.path.exists(so_path):
        return False
    lib = ctypes.CDLL(so_path)
    if not hasattr(lib, "axon_start_nrt_profile"):
        return False
    lib.axon_start_nrt_profile.argtypes = [
        ctypes.POINTER(ctypes.c_int64),
        ctypes.c_size_t,
    ]
    lib.axon_start_nrt_profile.restype = ctypes.c_int64
    lib.axon_stop_nrt_profile.argtypes = [ctypes.c_char_p]
    lib.axon_stop_nrt_profile.restype = ctypes.c_int64

    @contextlib.contextmanager
    def _hook(output_dir, device_ids):
        import jax
        jax.devices()
        if device_ids:
            ids = (ctypes.c_int64 * len(device_ids))(*device_ids)
            rc = lib.axon_start_nrt_profile(ids, len(device_ids))
        else:
            rc = lib.axon_start_nrt_profile(None, 0)
        if rc != 0:
            raise RuntimeError(f"axon_start_nrt_profile rc={rc}")
        try:
            yield
        finally:
            n = lib.axon_stop_nrt_profile(str(output_dir).encode())
            print(f"profile: {n} file(s) written to {output_dir}")

    import antenv
    mod = types.ModuleType("antenv.axon_hooks")
    mod._hook = _hook
    mod.get_axon_ntff_profile_hook = lambda: _hook
    mod.set_axon_ntff_profile_hook = lambda h: None
    _sys.modules["antenv.axon_hooks"] = mod
    antenv.axon_hooks = mod

    # artifact upload has no bucket access in this container; stub it
    import concourse.bass_utils as _bu
    _bu.upload_artifacts = lambda tmpdir: f"local://{tmpdir}"
    return True


def _classify_mask(mask):
    """Split the [S, S] additive mask into [t-128 x s-512] blocks per s-block.

    Returns (blocks, pats): blocks[m] = list of (j, pat_id|None) t-tiles to
    process for s-block m; pats = list of [128, 512] fp32 exp(mask) patterns.
    """
    mm = np.asarray(mask, np.float32).reshape(S, S)
    pats = []
    pat_ids = {}
    blocks = []
    for m in range(NSB):
        lst = []
        for j in range(NTT):
            blk = mm[m * 512:(m + 1) * 512, j * P:(j + 1) * P]  # [s, t]
            if np.all(blk <= -30.0):
                continue  # exp == 0: contributes nothing to av or den
            if np.all(blk == 0.0):
                lst.append((j, None))
                continue
            pt = np.exp(np.minimum(blk.T, 80.0)).astype(np.float32)  # [t, s]
            key = pt.tobytes()
            if key not in pat_ids:
                pat_ids[key] = len(pats)
                pats.append(pt)
            lst.append((j, pat_ids[key]))
        blocks.append(lst)
    return blocks, pats


def _emit(ctx, tc, io, blocks, npat):
    nc = tc.nc

    const = ctx.enter_context(tc.tile_pool(name="const", bufs=1))
    persist = ctx.enter_context(tc.tile_pool(name="persist", bufs=1))
    xt_pool = ctx.enter_context(tc.tile_pool(name="xt_pool", bufs=2))
    rope_pool = ctx.enter_context(tc.tile_pool(name="rope_pool", bufs=2))
    exp_pool = ctx.enter_context(tc.tile_pool(name="exp_pool", bufs=18))
    den_pool = ctx.enter_context(tc.tile_pool(name="den_pool", bufs=8))
    recip_pool = ctx.enter_context(tc.tile_pool(name="recip_pool", bufs=3))
    y_pool = ctx.enter_context(tc.tile_pool(name="y_pool", bufs=3))
    psum_mm = ctx.enter_context(tc.tile_pool(name="psum_mm", bufs=4, space="PSUM"))
    psum_acc = ctx.enter_context(tc.tile_pool(name="psum_acc", bufs=2, space="PSUM"))
    psum_den = ctx.enter_context(tc.tile_pool(name="psum_den", bufs=2, space="PSUM"))

    # --- constants / weights ---
    wq_sb = const.tile([P, NDT, HPC, P], BF16)
    nc.sync.dma_start(wq_sb[:], io["wqt"][:])
    wk_sb = const.tile([P, NDT, HPC, P], BF16)
    nc.sync.dma_start(wk_sb[:], io["wkt"][:])
    wv_sb = const.tile([P, NDT, EC], BF16)
    nc.sync.dma_start(wv_sb[:], io["wvt"][:])
    wo_sb = const.tile([P, HPC, D], BF16)
    cos_sb = const.tile([P, TOK], BF16)
    nc.sync.dma_start(cos_sb[:], io["cos2"][:])
    sin_sb = const.tile([P, TOK], BF16)
    nc.sync.dma_start(sin_sb[:], io["sin2"][:])
    pat_sb = const.tile([P, npat, 512], BF16)
    ones32 = const.tile([P, 32], BF16)
    nc.any.memset(ones32[:], 1.0)
    # selector: picks one representative row per 32-row column group and
    # sums them (bcast matmul lhsT), zeroing the redundant copies.
    sel4 = const.tile([P, P], BF16)
    nc.any.memset(sel4[:], 0.0)
    for g in range(4):
        nc.any.memset(sel4[ds(32 * g, 1), :], 1.0)

    q_sb = persist.tile([P, HPC, TOK], BF16)   # [parity*64+i, h, tok]
    k_sb = persist.tile([P, HPC, TOK], BF16)
    v_sb = persist.tile([P, TOK // P, EC], BF16)  # [t%128, t-tile, (h, j)]
    outT_sb = persist.tile([P, B * HPC, S], BF16)  # [j, pair, s]

    # --- phase 1: q/k/v projections + RoPE, per 512-token block ---
    for tb in range(NTB):
        xt_t = xt_pool.tile([P, NDT, 512], BF16)
        nc.sync.dma_start(xt_t[:], io["xt"][:, :, ts(tb, 512)])

        for w_sb, dst in ((wq_sb, q_sb), (wk_sb, k_sb)):
            for h in range(HPC):
                qk_ps = psum_mm.tile([P, 512], F32, tag="mm")
                for dt in range(NDT):
                    nc.tensor.matmul(
                        qk_ps[:], lhsT=w_sb[:, dt, h, :], rhs=xt_t[:, dt, :],
                        start=(dt == 0), stop=(dt == NDT - 1),
                    )
                nc.vector.tensor_copy(dst[:, h, ts(tb, 512)], qk_ps[:])

        for q4 in range(4):
            v_ps = psum_mm.tile([P, EC], F32, tag="mm")
            for dt in range(NDT):
                nc.tensor.matmul(
                    v_ps[:], lhsT=xt_t[:, dt, ts(q4, P)], rhs=wv_sb[:, dt, :],
                    start=(dt == 0), stop=(dt == NDT - 1),
                )
            nc.vector.tensor_copy(v_sb[:, tb * 4 + q4, :], v_ps[:])

        # RoPE on q and k for this token block (parity-major feature order:
        # partitions 0:64 hold even features t0, 64:128 hold odd t1).
        for a_sb in (q_sb, k_sb):
            swp = rope_pool.tile([P, HPC, 512], BF16, tag="swp")
            nc.sync.dma_start(swp[0:64, :, :], a_sb[64:128, :, ts(tb, 512)])
            nc.sync.dma_start(swp[64:128, :, :], a_sb[0:64, :, ts(tb, 512)])
            for h in range(HPC):
                r1 = rope_pool.tile([P, 512], BF16, tag="r1")
                nc.vector.tensor_mul(r1[:], a_sb[:, h, ts(tb, 512)], cos_sb[:, ts(tb, 512)])
                r2 = rope_pool.tile([P, 512], BF16, tag="r2")
                nc.vector.tensor_mul(r2[:], swp[:, h, :], sin_sb[:, ts(tb, 512)])
                nc.vector.tensor_add(a_sb[:, h, ts(tb, 512)], r1[:], r2[:])

    nc.sync.dma_start(pat_sb[:], io["pat"][:])

    # --- phase 2: attention per (batch, head) pair, per 512-query block ---
    den_rows = []
    for b in range(B):
        for h in range(HPC):
            pi = b * HPC + h
            for m in range(NSB):
                s_sl = ds(b * S + m * 512, 512)
                tlist = blocks[m]
                n_t = len(tlist)
                av_ps = psum_acc.tile([P, 512], F32, tag="acc")
                den4_ps = psum_den.tile([P, 512], F32, tag="den")
                # scores + exp first (PE runs ahead of ACT through the psum
                # pool); den/av matmuls afterwards so PE never waits on exp.
                exs = []
                for j, pid in tlist:
                    sc_ps = psum_mm.tile([P, 512], F32, tag="mm")
                    nc.tensor.matmul(
                        sc_ps[:], lhsT=k_sb[:, h, ds(b * S + j * P, P)],
                        rhs=q_sb[:, h, s_sl], start=True, stop=True,
                    )
                    ex = exp_pool.tile([P, 512], BF16, tag="ex")
                    nc.scalar.activation(ex[:], sc_ps[:], AF.Exp, scale=SCALE)
                    if pid is not None:
                        nc.vector.tensor_mul(ex[:], ex[:], pat_sb[:, pid, :])
                    exs.append(ex)
                # av matmuls per round of 4, then the 4 den matmuls issued
                # back-to-back into distinct PE column groups so they execute
                # concurrently (~1 matmul's time for all 4).
                nr = n_t // 4
                for r in range(nr):
                    for q4 in range(4):
                        idx = 4 * r + q4
                        j = tlist[idx][0]
                        nc.tensor.matmul(
                            av_ps[:], lhsT=v_sb[:, b * NTT + j, ds(h * HD, HD)],
                            rhs=exs[idx][:],
                            start=(idx == 0), stop=(idx == n_t - 1),
                        )
                    for q4 in range(4):
                        idx = 4 * r + q4
                        nc.tensor.matmul(
                            den4_ps[ds(32 * q4, 32), :], lhsT=ones32[:],
                            rhs=exs[idx][:], start=(r == 0), stop=(r == nr - 1),
                            tile_position=(0, 32 * q4),
                        )
                # evict UNNORMALIZED output; normalization is deferred off the
                # PE critical path so consecutive blocks' matmuls stay dense.
                den4_sb = den_pool.tile([P, 512], BF16, tag="denr")
                nc.vector.tensor_copy(den4_sb[:], den4_ps[:])
                nc.vector.tensor_copy(outT_sb[:, pi, ds(m * 512, 512)], av_ps[:])
                den_rows.append((pi, m, den4_sb))
            # normalize this pair's blocks; overlaps the next pair's attention
            for pi2, m2, d4 in den_rows:
                bc_ps = psum_mm.tile([P, 512], F32, tag="mm")
                nc.tensor.matmul(bc_ps[:], lhsT=sel4[:], rhs=d4[:],
                                 start=True, stop=True)
                rc = recip_pool.tile([P, 512], F32, tag="rc")
                nc.vector.reciprocal_approx_fast(rc[:], bc_ps[:])
                sl2 = ds(m2 * 512, 512)
                nc.vector.tensor_mul(outT_sb[:, pi2, sl2], outT_sb[:, pi2, sl2], rc[:])
            den_rows = []

    nc.sync.dma_start(wo_sb[:], io["wot"][:])

    # --- phase 3: o_proj partial: y[s, e] = sum_h outT_h^T @ woT_h ---
    for st in range(TOK // P):
        b = st // NTT
        sl = st % NTT
        for eb in range(D // 512):
            y_ps = psum_mm.tile([P, 512], F32, tag="mm")
            for h in range(HPC):
                nc.tensor.matmul(
                    y_ps[:], lhsT=outT_sb[:, b * HPC + h, ts(sl, P)],
                    rhs=wo_sb[:, h, ts(eb, 512)],
                    start=(h == 0), stop=(h == HPC - 1),
                )
            y_sb = y_pool.tile([P, 512], BF16, tag="y")
            if (st * (D // 512) + eb) % 2 == 0:
                nc.scalar.copy(y_sb[:], y_ps[:])
            else:
                nc.vector.tensor_copy(y_sb[:], y_ps[:])
            nc.sync.dma_start(io["y"][st, :, ts(eb, 512)], y_sb[:])


def _build_program(blocks_key, blocks, npat):
    nc = bacc.Bacc(
        "TRN2", target_bir_lowering=False, debug=False, enable_asserts=False
    )
    io = {
        "xt": nc.dram_tensor("xt", [P, NDT, TOK], BF16, kind="ExternalInput").ap(),
        "wqt": nc.dram_tensor("wqt", [P, NDT, HPC, P], BF16, kind="ExternalInput").ap(),
        "wkt": nc.dram_tensor("wkt", [P, NDT, HPC, P], BF16, kind="ExternalInput").ap(),
        "wvt": nc.dram_tensor("wvt", [P, NDT, EC], BF16, kind="ExternalInput").ap(),
        "wot": nc.dram_tensor("wot", [P, HPC, D], BF16, kind="ExternalInput").ap(),
        "cos2": nc.dram_tensor("cos2", [P, TOK], BF16, kind="ExternalInput").ap(),
        "sin2": nc.dram_tensor("sin2", [P, TOK], BF16, kind="ExternalInput").ap(),
        "pat": nc.dram_tensor("pat", [P, npat, 512], BF16, kind="ExternalInput").ap(),
        "y": nc.dram_tensor("y", [TOK // P, P, D], BF16, kind="ExternalOutput").ap(),
    }
    with tile.TileContext(nc) as tc:
        with ExitStack() as ctx:
            _emit(ctx, tc, io, blocks, npat)
    nc.compile()
    return nc


def _get_program(mask):
    blocks, pats = _classify_mask(mask)
    key = tuple(tuple(b) for b in blocks)
    if key not in _PROGRAM_CACHE:
        npat = max(len(pats), 1)
        nc = _build_program(key, blocks, npat)
        _PROGRAM_CACHE[key] = (nc, npat)
    nc, npat = _PROGRAM_CACHE[key]
    pat_np = np.zeros((P, npat, 512), np.float32)
    for i, pt in enumerate(pats):
        pat_np[:, i, :] = pt
    return nc, pat_np


def _bf16(a):
    return np.asarray(a, np.float32).astype(ml_dtypes.bfloat16)


def kernel(x, wq, wk, wv, wo, freqs_cos, freqs_sin, mask):
    global LAST_EXEC_NS
    x = np.asarray(x, np.float32)
    wq = np.asarray(wq, np.float32)
    wk = np.asarray(wk, np.float32)
    wv = np.asarray(wv, np.float32)
    wo = np.asarray(wo, np.float32)
    freqs_cos = np.asarray(freqs_cos, np.float32)
    freqs_sin = np.asarray(freqs_sin, np.float32)

    nc, pat_np = _get_program(mask)

    # xT: [d, tok] -> [dp, dt, tok]
    xt = _bf16(
        x.reshape(TOK, D).T.reshape(NDT, P, TOK).transpose(1, 0, 2)
    )

    # cos/sin, parity-major RoPE operands: [128, tok]
    cosT = np.tile(freqs_cos.T, (1, B))          # [64, TOK]
    sinT = np.tile(freqs_sin.T, (1, B))
    cos2 = _bf16(np.concatenate([cosT, cosT], axis=0))
    sin2 = _bf16(np.concatenate([-sinT, sinT], axis=0))
    pat = _bf16(pat_np)

    # per-head parity-major row permutation for q/k weights
    perm1 = np.r_[np.arange(0, P, 2), np.arange(1, P, 2)]

    in_maps = []
    for c in range(N_CORES):
        rows = slice(c * EC, (c + 1) * EC)
        wq_c, wk_c, wv_c = wq[rows], wk[rows], wv[rows]   # [256, D]
        wo_c = wo[:, rows]                                # [D, 256]
        row_perm = np.concatenate([h * P + perm1 for h in range(HPC)])
        wqt = _bf16(wq_c[row_perm].T.reshape(NDT, P, HPC, P).transpose(1, 0, 2, 3))
        wkt = _bf16(wk_c[row_perm].T.reshape(NDT, P, HPC, P).transpose(1, 0, 2, 3))
        wvt = _bf16(wv_c.T.reshape(NDT, P, EC).transpose(1, 0, 2))
        wot = _bf16(wo_c.T.reshape(HPC, P, D).transpose(1, 0, 2))
        in_maps.append({
            "xt": xt, "wqt": wqt, "wkt": wkt, "wvt": wvt, "wot": wot,
            "cos2": cos2, "sin2": sin2, "pat": pat,
        })

    if BACKEND == "sim":
        from concourse.bass_interp import CoreSim
        results = []
        for c in range(N_CORES):
            sim = CoreSim(nc, trace=False)
            for name, arr in in_maps[c].items():
                sim.tensor(name)[:] = arr
            sim.tensor("y")[:] = 0
            sim.simulate()
            results.append({"y": np.array(sim.tensor("y"))})
    else:
        do_trace = TRACE and _install_trace_hook()
        res = run_bass_kernel_spmd(
            nc, in_maps, core_ids=list(range(N_CORES)), trace=do_trace,
        )
        results = res.results
        LAST_EXEC_NS = res.exec_time_ns

    y = np.zeros((TOK, D), np.float32)
    for c in range(N_CORES):
        y += results[c]["y"].reshape(TOK, D).astype(np.float32)
    return y.reshape(B, S, D)

